# revision 26
# baseline (speedup 1.0000x reference)
"""Trainium2 Bass kernel for AgnosticChargeBiasedLinearPotentialEmbedding.

Math (per node n, for each irrep block l in {0,1,2} with multiplicity 128 and
m in 0..2l):
    out[n, off_l + o*(2l+1) + m] =
        (1/sqrt(128)) * sum_i node_feats[n, off_l + i*(2l+1) + m] * Wn_l[i, o]
        + potential_feats[n, poff_l + m] * Wp_l[0, o]
        + (l == 0) * local_charges[n, 0] * Wc0[0, o]

Device strategy (data-parallel over nodes, 8 cores, fp16 operands):
  - Host pre-transposes/deinterleaves node_feats into XT[i, lm, n] (fp16) so
    the TensorE streams node-columns against a stationary W_l — zero on-chip
    transposes, one weight load per (lm, pass).
  - Main matmul per (lm, 512-node chunk): out.T[o, n] = sum_i W_l[i, o] *
    X.T[i, n] into one PSUM bank.
  - Rank-1 (potential/charge) terms accumulate on top as a second matmul from
    a small per-node tensor, zero-padded to K=128 (K<32 matmuls don't count
    as PE activity for the HAM clock gate and leave the array at 1.2 GHz).
  - PSUM drained to SBUF (fp16) alternately by VectorE / ScalarE plain
    copies; stores go out transposed, superblock-contiguous; the host
    reassembles the natural [N, 1152] fp32 layout.
"""

import math

import numpy as np

import concourse.bass as bass
import concourse.tile as tile
from concourse import bacc, mybir
from concourse.bass_utils import run_bass_kernel_spmd

# Problem constants (hardcoded per contract; kernel.py must be self-contained).
N = 100000
N_CORES = 8
N_PER_CORE = 12544          # padded so 8 * 12544 = 100352 >= 100000
N_PAD = N_CORES * N_PER_CORE
LMS = [(0, 0), (1, 0), (1, 1), (1, 2), (2, 0), (2, 1), (2, 2), (2, 3), (2, 4)]
L_OFF = {0: 0, 1: 128, 2: 512}      # node-feats column offset of each l block
P_OFF = {0: 0, 1: 1, 2: 4}          # potential-feats column offset of each l
CHUNK = 512                  # matmul moving free dim (one PSUM bank of fp32)
SUPER = 2048                 # nodes per superblock (4 chunks -> 4 PSUM banks)
K_RANK1 = 128                # 9 potential rows + 1 charge row, zero-padded
                             # to full K so HAM sees full-array activity

IN_DT = mybir.dt.float16
OUT_DT = mybir.dt.float16
IN_NP = np.float16
OUT_NP = np.float16


def _superblocks():
    """(pos, size) tiling of N_PER_CORE; tapered at the end so the tail
    drains/stores (which nothing overlaps) come in finer pieces."""
    sizes = []
    rem = N_PER_CORE
    while rem > SUPER + 1024:
        sizes.append(SUPER)
        rem -= SUPER
    while rem > 0:
        sb = min(1024, rem)
        sizes.append(sb)
        rem -= sb
    supers = []
    pos = 0
    for sb in sizes:
        supers.append((pos, sb))
        pos += sb
    return supers


def _build_bass():
    nc = bacc.Bacc("TRN2", num_devices=N_CORES)

    # xt is packed superblock-major on the host: for each superblock the
    # [9, sb] block of every partition row is contiguous, so a superblock
    # load is 128 descriptors of 9*sb*2 contiguous bytes.
    xt = nc.declare_dram_parameter("xt", [128, 9 * N_PER_CORE], IN_DT, isOutput=False)
    w = nc.declare_dram_parameter("w", [128, 3, 128], IN_DT, isOutput=False)
    rw = nc.declare_dram_parameter("rw", [K_RANK1, 9, 128], IN_DT, isOutput=False)
    # Only the leading rows carry data (9 potential + 1 charge + pad to the
    # 32-partition alignment); rows 32..127 of the SBUF tile are zeroed once
    # on-chip instead of shipping ~2.5 MB of zeros per core over HBM.
    pt = nc.declare_dram_parameter("pt", [32, N_PER_CORE], IN_DT, isOutput=False)
    # out_t is packed superblock-major: for superblock at pos (size sb) the
    # (lm) block occupies the contiguous range 128*(9*pos + lm*sb) .. +128*sb,
    # laid out [128, sb] row-major, so every store is one contiguous region.
    out_t = nc.declare_dram_parameter("out_t", [9 * 128 * N_PER_CORE], OUT_DT, isOutput=True)

    supers = _superblocks()

    with tile.TileContext(nc) as tc:
        with (
            tc.tile_pool(name="const", bufs=1) as const_pool,
            tc.tile_pool(name="xw", bufs=3) as x_pool,
            tc.tile_pool(name="psum", bufs=2, space=bass.MemorySpace.PSUM) as psum_pool,
            tc.tile_pool(name="osb", bufs=6) as o_pool,
        ):
            # Resident constants (scalar/gpsimd rings so the first X-superblock
            # load on the sync ring starts immediately).
            pt_sb = const_pool.tile([K_RANK1, N_PER_CORE], IN_DT, tag="pt")
            nc.vector.memset(pt_sb[:], 0.0)
            nc.scalar.dma_start(pt_sb[0:32, :], pt[:])
            w_sb = const_pool.tile([128, 3, 128], IN_DT, tag="w")
            nc.scalar.dma_start(w_sb[:], w[:])
            rw_sb = const_pool.tile([K_RANK1, 9, 128], IN_DT, tag="rw")
            nc.scalar.dma_start(rw_sb[:], rw[:])

            drain_idx = 0
            for pos, sb in supers:
                xw = x_pool.tile([128, 9, sb], IN_DT, tag="xw")
                # Split the superblock load by lm-range so compute on the
                # first lm groups starts before the whole block lands and the
                # DMA interleaves more finely with stores.
                xt_sb_view = xt[:, 9 * pos:9 * (pos + sb)].rearrange(
                    "p (g n) -> p g n", g=9)
                for g0, g1 in ((0, 3), (3, 6), (6, 9)):
                    nc.sync.dma_start(xw[:, g0:g1, :], xt_sb_view[:, g0:g1, :])

                for lm, (l, _m) in enumerate(LMS):
                    ps = psum_pool.tile([128, sb], mybir.dt.float32, tag="ps")
                    # main pass first (one LDW, sb/CHUNK matmuls), then the
                    # rank-1 potential/charge pass accumulates on top (one
                    # LDW, sb/CHUNK matmuls). Grouping by stationary operand
                    # keeps LDW hidden; main-first lets superblock 0 start
                    # before the PT zero-fill completes.
                    for c0 in range(0, sb, CHUNK):
                        c1 = min(c0 + CHUNK, sb)
                        nc.tensor.matmul(
                            ps[:, c0:c1],
                            w_sb[:, l, :],
                            xw[:, lm, c0:c1],
                            start=True,
                            stop=False,
                        )
                    for c0 in range(0, sb, CHUNK):
                        c1 = min(c0 + CHUNK, sb)
                        nc.tensor.matmul(
                            ps[:, c0:c1],
                            rw_sb[:, lm, :],
                            pt_sb[:, pos + c0:pos + c1],
                            start=False,
                            stop=True,
                        )
                    osb = o_pool.tile([128, sb], OUT_DT, tag="osb")
                    if drain_idx % 2 == 0:
                        nc.vector.tensor_copy(osb[:], ps[:])
                    else:
                        nc.scalar.copy(osb[:], ps[:])
                    store_eng = nc.scalar if drain_idx % 2 == 0 else nc.gpsimd
                    drain_idx += 1
                    off = 128 * (9 * pos + lm * sb)
                    store_eng.dma_start(
                        out_t[off:off + 128 * sb].rearrange("(p n) -> p n", p=128),
                        osb[:])

    nc.compile()
    return nc


def _host_pack(potential_feats, node_feats, local_charges):
    """Build the device-side input tensors (all fp16)."""
    inv = 1.0 / math.sqrt(128.0)

    # XT[i, lm, n]: deinterleaved transpose of node_feats.
    xt = np.zeros((128, 9, N_PAD), dtype=IN_NP)
    for lm, (l, m) in enumerate(LMS):
        d = 2 * l + 1
        blk = node_feats[:, L_OFF[l] + m:L_OFF[l] + 128 * d:d]   # [N, 128]
        xt[:, lm, :N] = blk.T.astype(IN_NP)
    # Repack superblock-major per core: per partition row, each superblock's
    # [9, sb] block contiguous -> [128, 9*N_PER_CORE] per core.
    xt_sb = np.empty((128, N_CORES, 9 * N_PER_CORE), dtype=IN_NP)
    for c in range(N_CORES):
        base = c * N_PER_CORE
        for pos, sb in _superblocks():
            xt_sb[:, c, 9 * pos:9 * (pos + sb)] = (
                xt[:, :, base + pos:base + pos + sb].reshape(128, 9 * sb))
    xt = xt_sb

    # PT[k, n]: 9 potential rows (lm order) + charge row (+ zero pad to 32).
    ptm = np.zeros((32, N_PAD), dtype=IN_NP)
    for lm, (l, m) in enumerate(LMS):
        ptm[lm, :N] = potential_feats[:, P_OFF[l] + m].astype(IN_NP)
    ptm[9, :N] = local_charges[:, 0].astype(IN_NP)
    return xt, ptm, inv


def _host_weights(Wp0, Wp1, Wp2, Wn0, Wn1, Wn2, Wc0):
    inv = 1.0 / math.sqrt(128.0)
    w = np.stack([Wn0 * inv, Wn1 * inv, Wn2 * inv], axis=1).astype(IN_NP)  # [128,3,128]
    rw = np.zeros((K_RANK1, 9, 128), dtype=IN_NP)
    wp = {0: Wp0, 1: Wp1, 2: Wp2}
    for lm, (l, _m) in enumerate(LMS):
        rw[lm, lm, :] = wp[l][0].astype(IN_NP)
    rw[9, 0, :] = Wc0[0].astype(IN_NP)
    return w, rw


def _host_unpack(outs):
    """outs: list of 8 superblock-major flat arrays -> [N, 1152] fp32."""
    per_core = []
    for arr in outs:
        full_c = np.empty((9, 128, N_PER_CORE), dtype=arr.dtype)
        for pos, sb in _superblocks():
            base = 9 * 128 * pos
            full_c[:, :, pos:pos + sb] = arr[base:base + 9 * 128 * sb].reshape(9, 128, sb)
        per_core.append(full_c)
    full = np.concatenate(per_core, axis=2)    # [9, 128, N_PAD]
    out = np.empty((N, 1152), dtype=np.float32)
    lm = 0
    for l in (0, 1, 2):
        d = 2 * l + 1
        # rows lm..lm+d-1 -> [d, 128, N] -> natural [N, 128, d]
        blk = full[lm:lm + d, :, :N].astype(np.float32)
        out[:, L_OFF[l]:L_OFF[l] + 128 * d] = blk.transpose(2, 1, 0).reshape(N, 128 * d)
        lm += d
    return out


_NC_CACHE = {}


def _get_nc():
    if "nc" not in _NC_CACHE:
        _NC_CACHE["nc"] = _build_bass()
    return _NC_CACHE["nc"]


def _build_in_maps(potential_feats, node_feats, local_charges,
                   Wp0, Wp1, Wp2, Wn0, Wn1, Wn2, Wc0):
    xt, ptm, _ = _host_pack(potential_feats, node_feats, local_charges)
    w, rw = _host_weights(Wp0, Wp1, Wp2, Wn0, Wn1, Wn2, Wc0)
    in_maps = []
    for c in range(N_CORES):
        s = slice(c * N_PER_CORE, (c + 1) * N_PER_CORE)
        in_maps.append({
            "xt": np.ascontiguousarray(xt[:, c, :]),
            "w": w,
            "rw": rw,
            "pt": np.ascontiguousarray(ptm[:, s]),
        })
    return in_maps


def kernel(potential_feats, node_feats, node_attrs, local_charges,
           Wp0, Wp1, Wp2, Wn0, Wn1, Wn2, Wc0):
    del node_attrs  # explicitly unused in the reference forward
    in_maps = _build_in_maps(
        np.asarray(potential_feats, np.float32),
        np.asarray(node_feats, np.float32),
        np.asarray(local_charges, np.float32),
        np.asarray(Wp0, np.float32), np.asarray(Wp1, np.float32),
        np.asarray(Wp2, np.float32), np.asarray(Wn0, np.float32),
        np.asarray(Wn1, np.float32), np.asarray(Wn2, np.float32),
        np.asarray(Wc0, np.float32),
    )
    nc = _get_nc()
    res = run_bass_kernel_spmd(nc, in_maps, list(range(N_CORES)))
    outs = [res.results[c]["out_t"] for c in range(N_CORES)]
    return _host_unpack(outs)


# revision 27
# speedup vs baseline: 1.0233x; 1.0233x over previous
"""Trainium2 Bass kernel for AgnosticChargeBiasedLinearPotentialEmbedding.

Math (per node n, for each irrep block l in {0,1,2} with multiplicity 128 and
m in 0..2l):
    out[n, off_l + o*(2l+1) + m] =
        (1/sqrt(128)) * sum_i node_feats[n, off_l + i*(2l+1) + m] * Wn_l[i, o]
        + potential_feats[n, poff_l + m] * Wp_l[0, o]
        + (l == 0) * local_charges[n, 0] * Wc0[0, o]

Device strategy (data-parallel over nodes, 8 cores, fp16 operands):
  - Host pre-transposes/deinterleaves node_feats into XT[i, lm, n] (fp16) so
    the TensorE streams node-columns against a stationary W_l — zero on-chip
    transposes, one weight load per (lm, pass).
  - Main matmul per (lm, 512-node chunk): out.T[o, n] = sum_i W_l[i, o] *
    X.T[i, n] into one PSUM bank.
  - Rank-1 (potential/charge) terms accumulate on top as a second matmul from
    a small per-node tensor, zero-padded to K=128 (K<32 matmuls don't count
    as PE activity for the HAM clock gate and leave the array at 1.2 GHz).
  - PSUM drained to SBUF (fp16) alternately by VectorE / ScalarE plain
    copies; stores go out transposed, superblock-contiguous; the host
    reassembles the natural [N, 1152] fp32 layout.
"""

import math

import numpy as np

import concourse.bass as bass
import concourse.tile as tile
from concourse import bacc, mybir
from concourse.bass_utils import run_bass_kernel_spmd

# Problem constants (hardcoded per contract; kernel.py must be self-contained).
N = 100000
N_CORES = 8
N_PER_CORE = 12544          # padded so 8 * 12544 = 100352 >= 100000
N_PAD = N_CORES * N_PER_CORE
LMS = [(0, 0), (1, 0), (1, 1), (1, 2), (2, 0), (2, 1), (2, 2), (2, 3), (2, 4)]
L_OFF = {0: 0, 1: 128, 2: 512}      # node-feats column offset of each l block
P_OFF = {0: 0, 1: 1, 2: 4}          # potential-feats column offset of each l
CHUNK = 512                  # matmul moving free dim (one PSUM bank of fp32)
SUPER = 2048                 # nodes per superblock (4 chunks -> 4 PSUM banks)
K_RANK1 = 128                # 9 potential rows + 1 charge row, zero-padded
                             # to full K so HAM sees full-array activity

IN_DT = mybir.dt.float16
OUT_DT = mybir.dt.float16
IN_NP = np.float16
OUT_NP = np.float16


def _superblocks():
    """(pos, size) tiling of N_PER_CORE; tapered at both ends — small leading
    blocks so compute starts after a small load, small trailing blocks so the
    tail drains/stores (which nothing overlaps) come in finer pieces."""
    sizes = []
    rem = N_PER_CORE
    for lead in (512, 1024):
        if rem >= lead + SUPER:
            sizes.append(lead)
            rem -= lead
    while rem > SUPER + 1024:
        sizes.append(SUPER)
        rem -= SUPER
    while rem > 0:
        sb = min(1024, rem)
        sizes.append(sb)
        rem -= sb
    supers = []
    pos = 0
    for sb in sizes:
        supers.append((pos, sb))
        pos += sb
    return supers


def _build_bass():
    nc = bacc.Bacc("TRN2", num_devices=N_CORES)

    # xt is packed superblock-major on the host: for each superblock the
    # [9, sb] block of every partition row is contiguous, so a superblock
    # load is 128 descriptors of 9*sb*2 contiguous bytes.
    xt = nc.declare_dram_parameter("xt", [128, 9 * N_PER_CORE], IN_DT, isOutput=False)
    w = nc.declare_dram_parameter("w", [128, 3, 128], IN_DT, isOutput=False)
    rw = nc.declare_dram_parameter("rw", [K_RANK1, 9, 128], IN_DT, isOutput=False)
    # Only the leading rows carry data (9 potential + 1 charge + pad to the
    # 32-partition alignment); rows 32..127 of the SBUF tile are zeroed once
    # on-chip instead of shipping ~2.5 MB of zeros per core over HBM.
    pt = nc.declare_dram_parameter("pt", [32, N_PER_CORE], IN_DT, isOutput=False)
    # out_t is packed superblock-major in groups of 3 lm blocks: for the
    # superblock at pos (size sb), group g (lm 3g..3g+2) occupies the
    # contiguous range 128*(9*pos + g*3*sb) .. +128*3*sb laid out
    # [128 o, 3 lm, sb] row-major, so every store is one contiguous region
    # with 3*sb*2-byte per-partition chunks (matches the load chunking).
    out_t = nc.declare_dram_parameter("out_t", [9 * 128 * N_PER_CORE], OUT_DT, isOutput=True)

    supers = _superblocks()

    with tile.TileContext(nc) as tc:
        with (
            tc.tile_pool(name="const", bufs=1) as const_pool,
            tc.tile_pool(name="xw", bufs=3) as x_pool,
            tc.tile_pool(name="psum", bufs=2, space=bass.MemorySpace.PSUM) as psum_pool,
            tc.tile_pool(name="osb", bufs=6) as o_pool,
        ):
            # Resident constants (scalar/gpsimd rings so the first X-superblock
            # load on the sync ring starts immediately).
            pt_sb = const_pool.tile([K_RANK1, N_PER_CORE], IN_DT, tag="pt")
            nc.vector.memset(pt_sb[:], 0.0)
            nc.scalar.dma_start(pt_sb[0:32, :], pt[:])
            w_sb = const_pool.tile([128, 3, 128], IN_DT, tag="w")
            nc.scalar.dma_start(w_sb[:], w[:])
            rw_sb = const_pool.tile([K_RANK1, 9, 128], IN_DT, tag="rw")
            nc.scalar.dma_start(rw_sb[:], rw[:])

            drain_idx = 0
            for pos, sb in supers:
                xw = x_pool.tile([128, 9, sb], IN_DT, tag="xw")
                # Split the superblock load by lm-range so compute on the
                # first lm groups starts before the whole block lands and the
                # DMA interleaves more finely with stores.
                xt_sb_view = xt[:, 9 * pos:9 * (pos + sb)].rearrange(
                    "p (g n) -> p g n", g=9)
                for g0, g1 in ((0, 3), (3, 6), (6, 9)):
                    nc.sync.dma_start(xw[:, g0:g1, :], xt_sb_view[:, g0:g1, :])

                for lm, (l, _m) in enumerate(LMS):
                    ps = psum_pool.tile([128, sb], mybir.dt.float32, tag="ps")
                    # main pass first (one LDW, sb/CHUNK matmuls), then the
                    # rank-1 potential/charge pass accumulates on top (one
                    # LDW, sb/CHUNK matmuls). Grouping by stationary operand
                    # keeps LDW hidden; main-first lets superblock 0 start
                    # before the PT zero-fill completes.
                    for c0 in range(0, sb, CHUNK):
                        c1 = min(c0 + CHUNK, sb)
                        nc.tensor.matmul(
                            ps[:, c0:c1],
                            w_sb[:, l, :],
                            xw[:, lm, c0:c1],
                            start=True,
                            stop=False,
                        )
                    for c0 in range(0, sb, CHUNK):
                        c1 = min(c0 + CHUNK, sb)
                        nc.tensor.matmul(
                            ps[:, c0:c1],
                            rw_sb[:, lm, :],
                            pt_sb[:, pos + c0:pos + c1],
                            start=False,
                            stop=True,
                        )
                    g, gi = divmod(lm, 3)
                    if gi == 0:
                        osb = o_pool.tile([128, 3, sb], OUT_DT, tag="osb")
                    if drain_idx % 9 in (1, 3, 5, 7):   # 4/9 DVE, 5/9 ACT
                        nc.vector.tensor_copy(osb[:, gi, :], ps[:])
                    else:
                        nc.scalar.copy(osb[:, gi, :], ps[:])
                    drain_idx += 1
                    if gi == 2:
                        store_eng = nc.scalar if g % 2 == 0 else nc.gpsimd
                        off = 128 * (9 * pos + g * 3 * sb)
                        store_eng.dma_start(
                            out_t[off:off + 128 * 3 * sb].rearrange(
                                "(p g n) -> p g n", p=128, g=3),
                            osb[:])

    nc.compile()
    return nc


def _host_pack(potential_feats, node_feats, local_charges):
    """Build the device-side input tensors (all fp16)."""
    inv = 1.0 / math.sqrt(128.0)

    # XT[i, lm, n]: deinterleaved transpose of node_feats.
    xt = np.zeros((128, 9, N_PAD), dtype=IN_NP)
    for lm, (l, m) in enumerate(LMS):
        d = 2 * l + 1
        blk = node_feats[:, L_OFF[l] + m:L_OFF[l] + 128 * d:d]   # [N, 128]
        xt[:, lm, :N] = blk.T.astype(IN_NP)
    # Repack superblock-major per core: per partition row, each superblock's
    # [9, sb] block contiguous -> [128, 9*N_PER_CORE] per core.
    xt_sb = np.empty((128, N_CORES, 9 * N_PER_CORE), dtype=IN_NP)
    for c in range(N_CORES):
        base = c * N_PER_CORE
        for pos, sb in _superblocks():
            xt_sb[:, c, 9 * pos:9 * (pos + sb)] = (
                xt[:, :, base + pos:base + pos + sb].reshape(128, 9 * sb))
    xt = xt_sb

    # PT[k, n]: 9 potential rows (lm order) + charge row (+ zero pad to 32).
    ptm = np.zeros((32, N_PAD), dtype=IN_NP)
    for lm, (l, m) in enumerate(LMS):
        ptm[lm, :N] = potential_feats[:, P_OFF[l] + m].astype(IN_NP)
    ptm[9, :N] = local_charges[:, 0].astype(IN_NP)
    return xt, ptm, inv


def _host_weights(Wp0, Wp1, Wp2, Wn0, Wn1, Wn2, Wc0):
    inv = 1.0 / math.sqrt(128.0)
    w = np.stack([Wn0 * inv, Wn1 * inv, Wn2 * inv], axis=1).astype(IN_NP)  # [128,3,128]
    rw = np.zeros((K_RANK1, 9, 128), dtype=IN_NP)
    wp = {0: Wp0, 1: Wp1, 2: Wp2}
    for lm, (l, _m) in enumerate(LMS):
        rw[lm, lm, :] = wp[l][0].astype(IN_NP)
    rw[9, 0, :] = Wc0[0].astype(IN_NP)
    return w, rw


def _host_unpack(outs):
    """outs: list of 8 superblock-major flat arrays -> [N, 1152] fp32."""
    per_core = []
    for arr in outs:
        full_c = np.empty((9, 128, N_PER_CORE), dtype=arr.dtype)
        for pos, sb in _superblocks():
            base = 9 * 128 * pos
            seg = arr[base:base + 9 * 128 * sb].reshape(3, 128, 3, sb)
            full_c[:, :, pos:pos + sb] = seg.transpose(0, 2, 1, 3).reshape(9, 128, sb)
        per_core.append(full_c)
    full = np.concatenate(per_core, axis=2)    # [9, 128, N_PAD]
    out = np.empty((N, 1152), dtype=np.float32)
    lm = 0
    for l in (0, 1, 2):
        d = 2 * l + 1
        # rows lm..lm+d-1 -> [d, 128, N] -> natural [N, 128, d]
        blk = full[lm:lm + d, :, :N].astype(np.float32)
        out[:, L_OFF[l]:L_OFF[l] + 128 * d] = blk.transpose(2, 1, 0).reshape(N, 128 * d)
        lm += d
    return out


_NC_CACHE = {}


def _get_nc():
    if "nc" not in _NC_CACHE:
        _NC_CACHE["nc"] = _build_bass()
    return _NC_CACHE["nc"]


def _build_in_maps(potential_feats, node_feats, local_charges,
                   Wp0, Wp1, Wp2, Wn0, Wn1, Wn2, Wc0):
    xt, ptm, _ = _host_pack(potential_feats, node_feats, local_charges)
    w, rw = _host_weights(Wp0, Wp1, Wp2, Wn0, Wn1, Wn2, Wc0)
    in_maps = []
    for c in range(N_CORES):
        s = slice(c * N_PER_CORE, (c + 1) * N_PER_CORE)
        in_maps.append({
            "xt": np.ascontiguousarray(xt[:, c, :]),
            "w": w,
            "rw": rw,
            "pt": np.ascontiguousarray(ptm[:, s]),
        })
    return in_maps


def kernel(potential_feats, node_feats, node_attrs, local_charges,
           Wp0, Wp1, Wp2, Wn0, Wn1, Wn2, Wc0):
    del node_attrs  # explicitly unused in the reference forward
    in_maps = _build_in_maps(
        np.asarray(potential_feats, np.float32),
        np.asarray(node_feats, np.float32),
        np.asarray(local_charges, np.float32),
        np.asarray(Wp0, np.float32), np.asarray(Wp1, np.float32),
        np.asarray(Wp2, np.float32), np.asarray(Wn0, np.float32),
        np.asarray(Wn1, np.float32), np.asarray(Wn2, np.float32),
        np.asarray(Wc0, np.float32),
    )
    nc = _get_nc()
    res = run_bass_kernel_spmd(nc, in_maps, list(range(N_CORES)))
    outs = [res.results[c]["out_t"] for c in range(N_CORES)]
    return _host_unpack(outs)
